# revision 1
# baseline (speedup 1.0000x reference)
"""Multi-head causal attention with RoPE for TRN2, 8 NeuronCores.

Problem: B=2, T=2048, D=2048, 16 heads x head_dim 128, fp32.
  qkv = x @ Wqkv.T + bqkv ; RoPE(q, k) interleaved-pairs; causal softmax attention;
  out = attn_out @ Wo.T + bo.

Sharding: core c in 0..7 -> (batch b = c//4, head-group g = c%4 of 4 heads).
Each core computes its batch's partial output (its 4 heads' contribution through
the out-projection); host sums the 4 group partials per batch and adds bo.

Per-core kernel (all matmuls fp32r: full PE speed, ~1e-3 scale-relative rounding):
  Phase A: qkvT projection. q,k produced transposed [d, t] with head_dim rows
    de-interleaved (even dims then odd dims) so RoPE's rotate-half becomes a
    half-swap along partitions, done via a permutation matmul on PE. RoPE is
    applied during the PSUM drain. k lands directly in persistent SBUF tiles
    (no DRAM roundtrip); q and v roundtrip through DRAM.
  Phase B: per 512-wide q-tile, per head:
    S^T[k,q] = kT.T @ qT on PE, exp on ACT (scale 1/sqrt(dh) folded in),
    causal masks on diagonal blocks (DVE), PV accumulate O^T[d,q] (PE),
    denominator by ones-matmul (PE), reciprocal + partition-broadcast +
    normalize (DVE/GPSIMD).
  Phase C (interleaved per q-tile): final[t,o] += O^T_h[:,t].T @ WoT_h[:,o].
"""
import os
import sys

for _p in ("/opt/trn_rl_repo", "/root/.axon_site/_ro/trn_rl_repo"):
    if os.path.isdir(_p) and _p not in sys.path:
        sys.path.insert(0, _p)

import numpy as np

import concourse.bacc as bacc
import concourse.mybir as mybir
import concourse.tile as tile
from concourse.bass_utils import run_bass_kernel_spmd

dt = mybir.dt
AF = mybir.ActivationFunctionType

B = 2
T = 2048
D = 2048
NH = 16
HD = 128
ROPE_BASE = 10000.0
N_CORES = 8
GROUPS = 4          # head-groups (tensor-parallel axis)
HPG = NH // GROUPS  # heads per group = 4
FQK = HPG * HD      # 512: q (or k) feature cols per core
FV = HPG * HD       # 512
QT = 512            # q-tile width in attention
NQT = T // QT       # 4
NKC = T // 128      # 16 k-chunks
NCC = D // 128      # 16 contraction chunks
TB = 512            # phase-A t-block
NTB = T // TB       # 4
SCALE = 1.0 / float(np.sqrt(HD))


def build(loop=1):
    """Emit the per-core BIR program (identical for all 8 cores)."""
    import contextlib

    nc = bacc.Bacc("TRN2", target_bir_lowering=False, debug=False)

    xT_d = nc.dram_tensor("xT", [D, T], dt.float32r, kind="ExternalInput")
    wqp_d = nc.dram_tensor("wqpack", [8, 128, NCC * 128], dt.float32r,
                           kind="ExternalInput")
    wvp_d = nc.dram_tensor("wvpack", [128, NCC * FV], dt.float32r,
                           kind="ExternalInput")
    woT_d = nc.dram_tensor("woT", [FV, D], dt.float32r, kind="ExternalInput")
    cos_d = nc.dram_tensor("cosT", [HD, T], dt.float16, kind="ExternalInput")
    sin_d = nc.dram_tensor("sinT", [HD, T], dt.float16, kind="ExternalInput")
    mask_d = nc.dram_tensor("masks", [4, HD, QT], dt.float32, kind="ExternalInput")
    bqk_d = nc.dram_tensor("bqk", [2 * FQK, 1], dt.float32, kind="ExternalInput")
    bv_d = nc.dram_tensor("bvb", [HD, FV], dt.float32, kind="ExternalInput")
    ones_d = nc.dram_tensor("ones", [HD, 1], dt.float32r, kind="ExternalInput")
    perm_d = nc.dram_tensor("perm", [HD, HD], dt.float32r, kind="ExternalInput")
    out_d = nc.dram_tensor("outp", [T, D], dt.float32, kind="ExternalOutput")

    with tile.TileContext(nc, pool_alloc_mode="queue") as tc:
        with (
            tc.For_i(0, loop, 1) if loop > 1 else contextlib.nullcontext(),
            tc.tile_pool(name="dram", bufs=1, space="DRAM") as dramp,
            tc.tile_pool(name="kres", bufs=1) as kres,
        ):
            qT_tbs, vN_tbs = [], []
            for tb in range(NTB):
                qT_tb = dramp.tile([FQK, TB], dt.float32r, tag=f"qT{tb}",
                                   name=f"qT_{tb}")
                qT_tbs.append(qT_tb)
                vN_tb = dramp.tile([TB, FV], dt.float32r, tag=f"vN{tb}",
                                   name=f"vN_{tb}")
                vN_tbs.append(vN_tb)

            k_rs = []
            for h in range(HPG):
                k_r = kres.tile([HD, T], dt.float32r, tag=f"kr{h}", name=f"kr_{h}")
                k_rs.append(k_r)

            # -------- Phase A: qkv projection + RoPE on q,k (during drain) --------
            with (
                tc.tile_pool(name="wq", bufs=1) as wpool,
                tc.tile_pool(name="xb", bufs=2) as xpool,
                tc.tile_pool(name="adr", bufs=2) as adrain,
                tc.tile_pool(name="arope", bufs=2) as arope,
                tc.tile_pool(name="abias", bufs=1) as abias,
                tc.tile_pool(name="aps", bufs=2, space="PSUM") as aps,
                tc.tile_pool(name="rps", bufs=2, space="PSUM") as rps,
            ):
                bqk_sb = abias.tile([128, 8, 1], dt.float32)
                nc.scalar.dma_start(
                    out=bqk_sb, in_=bqk_d.ap().rearrange("(f p) o -> p f o", p=128)
                )
                bv_sb = abias.tile([HD, FV], dt.float32)
                nc.scalar.dma_start(out=bv_sb, in_=bv_d.ap())
                cos_t = abias.tile([HD, T], dt.float16)
                sin_t = abias.tile([HD, T], dt.float16)
                nc.scalar.dma_start(out=cos_t, in_=cos_d.ap())
                nc.scalar.dma_start(out=sin_t, in_=sin_d.ap())
                perm_t = abias.tile([HD, HD], dt.float32r)
                nc.scalar.dma_start(out=perm_t, in_=perm_d.ap())

                def load_xb(tb):
                    tsl = slice(tb * TB, (tb + 1) * TB)
                    xbl = []
                    for cc in range(NCC):
                        xb_c = xpool.tile(
                            [128, TB], dt.float32r, tag=f"xb{cc}",
                            name=f"xb_{tb}_{cc}", bufs=(1 if cc >= 14 else 2),
                        )
                        nc.sync.dma_start(
                            out=xb_c,
                            in_=xT_d.ap()[cc * 128:(cc + 1) * 128, tsl],
                        )
                        xbl.append(xb_c)
                    return xbl

                # first t-block's activations win the sync queue
                xb0 = load_xb(0)

                # qk weights as 8 column-blocks, split across both HWDGE
                # queues (evens+v on scalar, odds on sync behind xb0)
                wq_blocks = [None] * 8
                for fb, eng in ((0, nc.scalar), (4, nc.scalar), (1, nc.sync),
                                (5, nc.sync), (2, nc.scalar), (6, nc.scalar),
                                (3, nc.sync), (7, nc.sync)):
                    wq_b = wpool.tile([128, NCC, 128], dt.float32r, tag=f"wq{fb}",
                                      name=f"wq_{fb}")
                    eng.dma_start(
                        out=wq_b,
                        in_=wqp_d.ap()[fb].rearrange("p (cc f) -> p cc f", f=128),
                    )
                    wq_blocks[fb] = wq_b
                wv_b = wpool.tile([128, NCC, FV], dt.float32r)
                nc.scalar.dma_start(
                    out=wv_b,
                    in_=wvp_d.ap().rearrange("p (cc f) -> p cc f", f=FV),
                )
                for tb in range(NTB):
                    tsl = slice(tb * TB, (tb + 1) * TB)
                    xb = xb0 if tb == 0 else load_xb(tb)
                    # q,k: transposed [f, t]; RoPE during drain; k -> SBUF resident
                    for f in (0, 4, 1, 5, 2, 6, 3, 7):
                        ps = aps.tile([128, TB], dt.float32)
                        for cc in range(NCC):
                            nc.tensor.matmul(
                                ps,
                                wq_blocks[f][:, cc, :],
                                xb[cc],
                                start=(cc == 0),
                                stop=(cc == NCC - 1),
                            )
                        s1 = arope.tile([128, TB], dt.float32r, tag="s1")
                        nc.vector.tensor_scalar_add(s1, ps, bqk_sb[:, f, :])
                        rot_ps = rps.tile([128, TB], dt.float32)
                        nc.tensor.matmul(rot_ps, perm_t, s1, start=True, stop=True)
                        nc.vector.tensor_mul(out=s1, in0=s1, in1=cos_t[:, tsl])
                        nc.vector.tensor_mul(out=rot_ps, in0=rot_ps, in1=sin_t[:, tsl])
                        if f < 4:  # q -> DRAM roundtrip
                            dr = adrain.tile([128, TB], dt.float32r, tag="adr")
                            nc.vector.tensor_add(out=dr, in0=s1, in1=rot_ps)
                            nc.sync.dma_start(
                                out=qT_tbs[tb][f * 128:(f + 1) * 128, :], in_=dr,
                            )
                        else:      # k -> persistent SBUF
                            nc.vector.tensor_add(
                                out=k_rs[f - 4][:, tsl], in0=s1, in1=rot_ps
                            )
                    # v: natural output [t, d]
                    for ts4 in range(TB // 128):
                        ps = aps.tile([128, FV], dt.float32)
                        for cc in range(NCC):
                            nc.tensor.matmul(
                                ps,
                                xb[cc][:, ts4 * 128:(ts4 + 1) * 128],
                                wv_b[:, cc, :],
                                start=(cc == 0),
                                stop=(cc == NCC - 1),
                            )
                        dr = adrain.tile([128, FV], dt.float32r, tag="adr")
                        nc.vector.tensor_add(dr, ps, bv_sb)
                        nc.sync.dma_start(
                            out=vN_tbs[tb][ts4 * 128:(ts4 + 1) * 128, :],
                            in_=dr,
                        )

            # -------- Phase B + C: attention, out-proj per q-tile --------
            with (
                tc.tile_pool(name="bsing", bufs=1) as bsing,
                tc.tile_pool(name="qt", bufs=2) as qtp,
                tc.tile_pool(name="vt", bufs=1) as vtp,
                tc.tile_pool(name="osb", bufs=2) as osbp,
                tc.tile_pool(name="pt", bufs=4) as ptp,
                tc.tile_pool(name="bsmall", bufs=2) as bsmall,
                tc.tile_pool(name="wo", bufs=1) as wop,
                tc.tile_pool(name="cdr", bufs=3) as cdrain,
                tc.tile_pool(name="ps_s", bufs=3, space="PSUM") as ps_s,
                tc.tile_pool(name="ps_o", bufs=2, space="PSUM") as ps_o,
                tc.tile_pool(name="ps_l", bufs=1, space="PSUM") as ps_l,
                tc.tile_pool(name="cps", bufs=2, space="PSUM") as cps,
            ):
                mask_t = bsing.tile([HD, 4, QT], dt.float32)
                nc.sync.dma_start(out=mask_t, in_=mask_d.ap().transpose([1, 0, 2]))
                ones_t = bsing.tile([HD, 1], dt.float32r)
                nc.scalar.dma_start(out=ones_t, in_=ones_d.ap())

                # first q-tile's q loads win the queue; v chunks tb-major,
                # alternating the two HWDGE queues
                q_t0s = []
                for h in range(HPG):
                    q_t = qtp.tile([HD, QT], dt.float32r, tag=f"qt{h}",
                                   name=f"qt_0_{h}")
                    nc.scalar.dma_start(out=q_t, in_=qT_tbs[0][h * HD:(h + 1) * HD, :])
                    q_t0s.append(q_t)
                v_ts = []
                for h in range(HPG):
                    v_t = vtp.tile([128, NKC, HD], dt.float32r, tag=f"v{h}",
                                   name=f"v_{h}")
                    v_ts.append(v_t)
                qi = 0
                for tb in range(NTB):
                    for h in range(HPG):
                        eng = nc.scalar if (qi % 2 == 0) else nc.sync
                        qi += 1
                        eng.dma_start(
                            out=v_ts[h][:, 4 * tb:4 * (tb + 1), :],
                            in_=vN_tbs[tb][:, h * HD:(h + 1) * HD].rearrange(
                                "(c p) d -> p c d", p=128
                            ),
                        )

                wo_sb = wop.tile([128, HPG, D], dt.float32r)
                nc.sync.dma_start(
                    out=wo_sb, in_=woT_d.ap().rearrange("(hh p) o -> p hh o", p=128)
                )

                def emit_cproj(pj, o_hs, tts, on_act=False):
                    # out-projection tiles (tt in tts) for q-tile pj
                    for tt in tts:
                        for oo in range(D // QT):
                            ps = cps.tile([128, QT], dt.float32,
                                          name=f"cps_{pj}_{tt}_{oo}", tag="cps")
                            for h in range(HPG):
                                nc.tensor.matmul(
                                    ps,
                                    o_hs[h][:, tt * 128:(tt + 1) * 128],
                                    wo_sb[:, h, oo * QT:(oo + 1) * QT],
                                    start=(h == 0), stop=(h == HPG - 1),
                                )
                            dr = cdrain.tile([128, QT], dt.float32,
                                             name=f"cdr_{pj}_{tt}_{oo}", tag="cdr")
                            if on_act:
                                nc.scalar.copy(out=dr, in_=ps)
                            else:
                                nc.vector.tensor_copy(out=dr, in_=ps)
                            nc.sync.dma_start(
                                out=out_d.ap()[
                                    pj * QT + tt * 128: pj * QT + (tt + 1) * 128,
                                    oo * QT:(oo + 1) * QT,
                                ],
                                in_=dr,
                            )

                prev_o = None
                for j in range(NQT):
                    nkc = 4 * (j + 1)
                    o_heads = []
                    for h in range(HPG):
                        if j == 0:
                            q_t = q_t0s[h]
                        else:
                            q_t = qtp.tile([HD, QT], dt.float32r, tag=f"qt{h}",
                                           name=f"qt_{j}_{h}")
                            nc.scalar.dma_start(
                                out=q_t, in_=qT_tbs[j][h * HD:(h + 1) * HD, :]
                            )
                        o_head_tile = osbp.tile([HD, QT], dt.float32r, tag=f"osb{h}",
                                                name=f"osb_{j}_{h}")
                        o_heads.append(o_head_tile)
                        psum_o = ps_o.tile([HD, QT], dt.float32)
                        psum_l = ps_l.tile([1, QT], dt.float32)

                        def col0(kc):
                            m = kc - 4 * j
                            if m <= 0:
                                return 0
                            return 128 if m == 1 else 256

                        def s_matmul(kc):
                            c0 = col0(kc)
                            psum_s = ps_s.tile(
                                [128, QT], dt.float32,
                                name=f"s_{j}_{h}_{kc}", tag="psum_s",
                            )
                            nc.tensor.matmul(
                                psum_s[:, c0:],
                                k_rs[h][:, kc * 128:(kc + 1) * 128],
                                q_t[:, c0:],
                                start=True, stop=True,
                            )
                            return psum_s

                        s_next = s_matmul(0)
                        for kc in range(nkc):
                            psum_s = s_next
                            if kc + 1 < nkc:
                                s_next = s_matmul(kc + 1)
                            c0 = col0(kc)
                            pt = ptp.tile([128, QT], dt.float32r)
                            nc.scalar.activation(
                                out=pt[:, c0:], in_=psum_s[:, c0:],
                                func=AF.Exp, scale=SCALE,
                            )
                            m = kc - 4 * j
                            if m >= 0:
                                nc.vector.tensor_mul(
                                    out=pt[:, c0:], in0=pt[:, c0:],
                                    in1=mask_t[:, m, c0:],
                                )
                            nc.tensor.matmul(
                                psum_o[:, c0:], v_ts[h][:, kc, :], pt[:, c0:],
                                start=(kc == 0), stop=(kc == nkc - 1),
                            )
                            nc.tensor.matmul(
                                psum_l[:, c0:], ones_t, pt[:, c0:],
                                start=(kc == 0), stop=(kc == nkc - 1),
                            )
                        recip = bsmall.tile([1, QT], dt.float32, tag="recip")
                        nc.vector.reciprocal(out=recip, in_=psum_l)
                        bcast = bsmall.tile([128, QT], dt.float32, tag="bcast")
                        nc.gpsimd.partition_broadcast(bcast, recip)
                        nc.vector.tensor_mul(
                            out=o_heads[h], in0=psum_o, in1=bcast
                        )
                        # interleave previous q-tile's out-projection
                        if prev_o is not None:
                            emit_cproj(j - 1, prev_o, [h])
                    prev_o = o_heads
                emit_cproj(NQT - 1, prev_o, list(range(QT // 128)), on_act=True)
    nc.compile()
    return nc


# ---------------------------------------------------------------------------
# Host side
# ---------------------------------------------------------------------------

_DEINT = np.concatenate([np.arange(0, HD, 2), np.arange(1, HD, 2)])  # de-interleave


def _rope_tables():
    half = HD // 2
    inv_freq = 1.0 / (ROPE_BASE ** (np.arange(half, dtype=np.float64) / half))
    t = np.arange(T, dtype=np.float64)
    fr = t[None, :] * inv_freq[:, None]          # (64, T)
    cos = np.concatenate([np.cos(fr), np.cos(fr)], axis=0).astype(np.float16)
    sin = np.concatenate([-np.sin(fr), np.sin(fr)], axis=0).astype(np.float16)
    return cos, sin


def _masks():
    m = np.zeros((4, HD, QT), dtype=np.float32)
    kk = np.arange(HD)[:, None]
    qq = np.arange(QT)[None, :]
    for i in range(4):
        m[i] = (kk <= qq - 128 * i).astype(np.float32)
    return m


def _perm():
    p = np.zeros((HD, HD), dtype=np.float32)
    half = HD // 2
    for i in range(half):
        p[i + half, i] = 1.0   # rot[m<64]  = s1[m+64]
        p[i, i + half] = 1.0   # rot[m>=64] = s1[m-64]
    return p


def make_in_maps(x, Wqkv, bqkv, Wo, bo):
    cos, sin = _rope_tables()
    masks = _masks()
    ones = np.ones((HD, 1), dtype=np.float32)
    perm = _perm()

    Wq = Wqkv[0 * D:1 * D]
    Wk = Wqkv[1 * D:2 * D]
    Wv = Wqkv[2 * D:3 * D]
    bq = bqkv[0 * D:1 * D]
    bk = bqkv[1 * D:2 * D]
    bv = bqkv[2 * D:3 * D]

    in_maps = []
    for c in range(N_CORES):
        b, g = divmod(c, GROUPS)
        hsl = slice(g * HPG * HD, (g + 1) * HPG * HD)
        # de-interleaved row order for q,k heads of this group
        rows = np.arange(g * HPG * HD, (g + 1) * HPG * HD).reshape(HPG, HD)
        rows = rows[:, _DEINT].reshape(-1)

        wq = Wq[rows]                       # (512, D)
        wk = Wk[rows]
        wv = Wv[hsl]                        # natural order
        wqkT = np.concatenate([wq, wk], axis=0).T.astype(np.float32)  # (D, 1024)
        # packed [fb, p, cc*f]: per-partition contiguous DMA rows
        wqpack = np.ascontiguousarray(
            wqkT.reshape(NCC, 128, 8, 128)      # (cc, p, fb, f)
                .transpose(2, 1, 0, 3)           # (fb, p, cc, f)
                .reshape(8, 128, NCC * 128)
        )
        wvT = wv.T.astype(np.float32)            # (D, 512)
        wvpack = np.ascontiguousarray(
            wvT.reshape(NCC, 128, FV).transpose(1, 0, 2).reshape(128, NCC * FV)
        )
        woT = np.ascontiguousarray(Wo[:, hsl].T.astype(np.float32))  # (512, D)

        bqk = np.concatenate([bq[rows], bk[rows]]).astype(np.float32)[:, None]
        bvb = np.broadcast_to(bv[hsl].astype(np.float32), (HD, FV)).copy()

        xT = np.ascontiguousarray(np.asarray(x[b]).T.astype(np.float32))  # (D, T)

        in_maps.append({
            "xT": xT,
            "wqpack": wqpack,
            "wvpack": wvpack,
            "woT": woT,
            "cosT": cos,
            "sinT": sin,
            "masks": masks,
            "bqk": bqk,
            "bvb": bvb,
            "ones": ones,
            "perm": perm,
        })
    return in_maps


_NC_CACHE = {}


def _get_nc(loop=1):
    if loop not in _NC_CACHE:
        _NC_CACHE[loop] = build(loop=loop)
    return _NC_CACHE[loop]


def kernel(x, Wqkv, bqkv, Wo, bo):
    x = np.asarray(x)
    Wqkv = np.asarray(Wqkv)
    bqkv = np.asarray(bqkv)
    Wo = np.asarray(Wo)
    bo = np.asarray(bo)

    nc = _get_nc()
    in_maps = make_in_maps(x, Wqkv, bqkv, Wo, bo)
    res = run_bass_kernel_spmd(nc, in_maps, core_ids=list(range(N_CORES)))

    out = np.zeros((B, T, D), dtype=np.float32)
    for c in range(N_CORES):
        b = c // GROUPS
        out[b] += res.results[c]["outp"]
    out += bo.astype(np.float32)[None, None, :]
    return out

